# revision 32
# baseline (speedup 1.0000x reference)
"""Trainium2 Bass kernel for nn_DifferentiablePersistence.

Math: betti_0(t) = sum_i exp(-lambda_i(L_t)/sigma) = trace(expm(-L_t/sigma)),
so no eigensolver is needed -- a scaling-and-squaring matrix exponential
(pure 768^3 fp32 matmuls on the tensor engine) computes the trace:

    A     = -(L/sigma) / 2^s           (symmetric NSD, ||A|| <= THETA)
    B     = taylor_16(A)               (Paterson-Stockmeyer: 6 matmuls,
                                        coefficient blocks on the vector engine)
    B    <- B^2, (s-1) times           (chained squarings)
    betti = ||B||_F^2                  (= trace(B^(2^s)), last squaring free)

Host-side spectral triage (power iterations, no eigensolves): thresholds
whose algebraic connectivity lam_2 >= 3 have betti = 1 + 767*exp(-30) --
exactly 1 to 7 digits -- and skip the device entirely.  The remaining
(~13) thresholds are LPT-balanced over 8 cores x ceil(K/8) slots.  The
per-threshold squaring count s (from lambda_max(L_t)) is runtime data
driving a register-bound hardware loop, so one SPMD NEFF serves every
threshold with its exact s.  The distance matrix is replicated; each core
returns per-slot Frobenius partials (ACT Square + per-channel accumulate)
which the host reduces and feeds through the cheap (5,100) landscape
post-processing.
"""

import math
import os

import numpy as np

SIGMA = 0.1
RESOLUTION = 100
NUM_LANDSCAPES = 5
NUM_THRESHOLDS = 50
N = 768
P = 128
KO = N // P          # 6 k-subtiles
NCHUNK = 384         # matmul free-dim chunk (>=256 keeps float32r at rate 1.0)
NCORES = 8
SLOTS = 7
DEG = 16             # Taylor degree (Paterson-Stockmeyer blocks of 4)
THETA = 1.8          # max ||A|| after scaling (deg-16 remainder < 1e-10)

# build-time switches (env overrides for experiments)
USE_FP32R = os.environ.get("KB_FP32R", "0") == "1"
MAX_DSQ = 8          # upper bound on double-squaring trip count (s <= 17)
LAM2_TRIVIAL = 3.0   # lam_2 above this => betti = 1 + O(1e-10), skip device work
ALL_DEVICE = os.environ.get("KB_ALL_DEVICE", "0") == "1"

_COMPILED = {}


# ----------------------------------------------------------------- host math

def _compute_dist(points):
    """fp32 pairwise distances exactly like the jax reference."""
    pts = points.astype(np.float32)
    diff = pts[:, None, :] - pts[None, :, :]
    d2 = (diff * diff).sum(-1, dtype=np.float32)
    dist = np.where(d2 > 0, np.sqrt(np.where(d2 > 0, d2, np.float32(1.0))), np.float32(0.0))
    return dist.astype(np.float32)


def _analyze(dist, thresholds):
    """Per-threshold spectral analysis via power iterations (host, cheap):

    - lam_max(L_t)  -> squaring count s_t
    - lam_2(L_t)    -> thresholds with lam_2 >= LAM2_TRIVIAL are 'trivial':
      betti-1 = sum_{i>=2} exp(-lam_i/sigma) <= N*exp(-LAM2_TRIVIAL/sigma)
      < 2e-7, so betti := 1 exactly (the lam=0 constant mode survives).

    Returns (s_arr, trivial_mask)."""
    T = len(thresholds)
    d = dist.astype(np.float32)
    S = 1.0 / (1.0 + np.exp(-(thresholds[:, None, None].astype(np.float32) - d) / np.float32(SIGMA)))
    deg = S.sum(-1)                                     # (T, N)

    def power_lmax(niter=40):
        v = deg / np.linalg.norm(deg, axis=-1, keepdims=True)
        lam = np.zeros(T)
        for _ in range(niter):
            w = deg * v - np.einsum("tij,tj->ti", S, v)  # L v
            lam = np.abs((v * w).sum(-1))
            v = w / np.maximum(np.linalg.norm(w, axis=-1, keepdims=True), 1e-30)
        return lam

    lam_max = power_lmax()
    lub = lam_max * 1.08 + 1e-6
    a = lub / SIGMA
    s = np.ceil(np.log2(np.maximum(a / THETA, 1.0))).astype(np.int64)
    s = np.maximum(s, 1)
    s = np.minimum(s, 2 * MAX_DSQ + 1)

    # lam_2 via power iteration on M = lub*I - L restricted to 1-perp
    rng = np.random.default_rng(12345)
    lam2_ests = []
    for _ in range(2):
        v = rng.standard_normal((T, dist.shape[0])).astype(np.float64)
        v -= v.mean(-1, keepdims=True)
        v /= np.linalg.norm(v, axis=-1, keepdims=True)
        top = np.zeros(T)
        for _ in range(80):
            Lv = deg * v - np.einsum("tij,tj->ti", S, v)
            w = lub[:, None] * v - Lv                    # M v
            w -= w.mean(-1, keepdims=True)               # project out constant
            top = (v * w).sum(-1)
            v = w / np.maximum(np.linalg.norm(w, axis=-1, keepdims=True), 1e-30)
        lam2_ests.append(lub - top)                      # >= lam_2 (upper est)
    lam2 = np.minimum(*lam2_ests)
    trivial = lam2 >= LAM2_TRIVIAL
    return s, trivial


def _assign(active, s_arr, slots):
    """LPT-balance the active thresholds + duplicate pad items onto
    (core, slot) by per-item cost (DEG-1 Horner matmuls + s squarings).

    Returns assign[core][slot] = threshold index."""
    npad = NCORES * slots - len(active)
    cheap = sorted(active, key=lambda t: s_arr[t])[:npad]
    while len(cheap) < npad:  # more pads than actives: repeat
        cheap = (cheap + cheap)[:npad]
    items = list(active) + cheap
    items.sort(key=lambda t: -s_arr[t])
    loads = [0.0] * NCORES
    counts = [0] * NCORES
    assign = [[] for _ in range(NCORES)]
    for t in items:
        c = min(
            (c for c in range(NCORES) if counts[c] < slots),
            key=lambda c: (loads[c], counts[c]),
        )
        assign[c].append(int(t))
        loads[c] += (DEG - 1) + int(s_arr[t])
        counts[c] += 1
    return assign


def _landscapes(betti_0):
    """Replicate the reference post-processing (host side, float64)."""
    x = betti_0.astype(np.float64)
    t = x.shape[0]
    pos = np.linspace(0.0, t - 1.0, RESOLUTION)
    i0 = np.clip(np.floor(pos).astype(np.int64), 0, t - 2)
    frac = pos - i0
    bi = x[i0] * (1.0 - frac) + x[i0 + 1] * frac
    out = [bi / (bi.max() + 1e-8)]
    for k in range(1, NUM_LANDSCAPES):
        ks = min(2 * k + 1, RESOLUTION // 4)
        if ks > 1:
            pad = ks // 2
            padded = np.pad(bi, (pad, pad), mode="edge")
            sm = np.convolve(padded, np.ones(ks) / ks, mode="valid")
            dv = sm[1:] - sm[:-1]
            dv = np.concatenate([dv, dv[-1:]])
            out.append(dv / (np.abs(dv).max() + 1e-8))
        else:
            out.append(out[0])
    return np.stack(out).astype(np.float32)


# -------------------------------------------------------------- bass kernel

def _build_nc(slots):
    import concourse.bass as bass
    import concourse.mybir as mybir
    import concourse.tile as tile
    from concourse import bacc
    from concourse.masks import make_identity

    f32 = mybir.dt.float32
    dt_mm = mybir.dt.float32r if USE_FP32R else mybir.dt.float32

    nc = bacc.Bacc("TRN2", target_bir_lowering=False)
    dist_d = nc.declare_dram_parameter("dist", [P, KO * N], f32, isOutput=False)
    bias_d = nc.declare_dram_parameter("bias", [P, slots], f32, isOutput=False)
    qs_d = nc.declare_dram_parameter("qs", [P, slots], f32, isOutput=False)
    nsq_d = nc.declare_dram_parameter("nsq", [1, slots], mybir.dt.int32, isOutput=False)
    npar_d = nc.declare_dram_parameter("npar", [1, slots], mybir.dt.int32, isOutput=False)
    nrep_d = nc.declare_dram_parameter("nrep", [1, 1], mybir.dt.int32, isOutput=False)
    fro_d = nc.declare_dram_parameter("fro", [P, slots], f32, isOutput=True)

    coef = [1.0 / math.factorial(k) for k in range(DEG + 1)]

    with tile.TileContext(nc) as tc:
        with (
            tc.tile_pool(name="const", bufs=1) as constp,
            tc.tile_pool(name="mats", bufs=1) as matp,
            tc.tile_pool(name="sq", bufs=2) as sqp,
            tc.tile_pool(name="small", bufs=2) as smallp,
            tc.tile_pool(name="ps", bufs=8, space="PSUM") as psp,
        ):
            dist_sb = constp.tile([P, KO, N], f32, tag="dist")
            nc.gpsimd.dma_start(dist_sb[:], dist_d.ap().rearrange("p (ko f) -> p ko f", ko=KO))
            bias_sb = constp.tile([P, slots], f32, tag="bias")
            nc.gpsimd.dma_start(bias_sb[:], bias_d.ap())
            qs_sb = constp.tile([P, slots], f32, tag="qs")
            nc.gpsimd.dma_start(qs_sb[:], qs_d.ap())
            nsq_sb = constp.tile([1, slots], mybir.dt.int32, tag="nsq")
            nc.gpsimd.dma_start(nsq_sb[:], nsq_d.ap())
            npar_sb = constp.tile([1, slots], mybir.dt.int32, tag="npar")
            nc.gpsimd.dma_start(npar_sb[:], npar_d.ap())
            nrep_sb = constp.tile([1, 1], mybir.dt.int32, tag="nrep")
            nc.gpsimd.dma_start(nrep_sb[:], nrep_d.ap())

            ident = constp.tile([P, P], f32, tag="ident")
            make_identity(nc, ident[:])
            cid = []
            for k in range(DEG):                      # scaled identities coef[0..11]
                ck = constp.tile([P, P], f32, tag=f"cid{k}")
                nc.vector.tensor_scalar_mul(ck[:], ident[:], coef[k])
                cid.append(ck)

            fro_sb = constp.tile([P, slots], f32, tag="fro")

            def mm_group(dst, lhs, rhs, addmat=None):
                """dst = lhs @ rhs (+ addmat) -- [P, KO, N] fp32 SBUF, symmetric
                operands; evacuation alternates DVE/ACT, or does a DVE
                PSUM+SBUF add when addmat is given."""
                piece = 0
                for m in range(KO):
                    for n0 in range(0, N, NCHUNK):
                        pt = psp.tile([P, NCHUNK], f32, tag="ps")
                        for k in range(KO):
                            nc.tensor.matmul(
                                pt[:],
                                lhs[:, k, m * P : (m + 1) * P],
                                rhs[:, k, n0 : n0 + NCHUNK],
                                start=(k == 0),
                                stop=(k == KO - 1),
                            )
                        out_ap = dst[:, m, n0 : n0 + NCHUNK]
                        if addmat is not None:
                            nc.vector.tensor_tensor(
                                out_ap, pt[:], addmat[:, m, n0 : n0 + NCHUNK],
                                mybir.AluOpType.add,
                            )
                        elif piece % 2 == 0:
                            nc.vector.tensor_copy(out_ap, pt[:])
                        else:
                            nc.scalar.copy(out_ap, pt[:])
                        piece += 1

            def diag_view(mat):
                """[P, KO, P] view of mat's 128-block diagonal: (p, ko, f) ->
                free offset ko*(N+P) + f."""
                t = mat[:]
                return bass.AP(t.tensor, t.offset, [[KO * N, P], [N + P, KO], [1, P]])

            def diag_add(mat, ck):
                """mat[i, i] += ck[i % P, i % P] (per 128-block diagonal)."""
                dv = diag_view(mat)
                nc.vector.tensor_tensor(
                    dv,
                    dv,
                    ck[:, None, :].to_broadcast([P, KO, P]),
                    mybir.AluOpType.add,
                )

            rep_regs = []
            for e in mybir.ALL_ENGINES:
                r = nc.alloc_register(e, f"nrep_{e.name}")
                nc.engines[e].reg_load(r, nrep_sb[:1, :1])
                rep_regs.append(r)
            n_rep = bass.make_scalar_value(
                bass.RegisterHandles(rep_regs), min_val=1, max_val=1000000
            )

            with tc.For_i(0, n_rep, 1):
                for j in range(slots):
                    # ---- per-slot registers: double-squaring trip count
                    regs = []
                    for e in mybir.ALL_ENGINES:
                        r = nc.alloc_register(e, f"nsq_{j}_{e.name}")
                        nc.engines[e].reg_load(r, nsq_sb[:1, j : j + 1])
                        regs.append(r)
                    n_dsq = bass.make_scalar_value(
                        bass.RegisterHandles(regs), min_val=0, max_val=MAX_DSQ
                    )
                    regs_p = []
                    for e in mybir.ALL_ENGINES:
                        r = nc.alloc_register(e, f"npar_{j}_{e.name}")
                        nc.engines[e].reg_load(r, npar_sb[:1, j : j + 1])
                        regs_p.append(r)
                    n_par = bass.make_scalar_value(
                        bass.RegisterHandles(regs_p), min_val=0, max_val=1
                    )

                    # ---- A = qs * (S - diag(deg));  S = sigmoid(-dist/sigma + t/sigma)
                    A = matp.tile([P, KO, N], dt_mm, tag="A")
                    nc.scalar.activation(
                        A[:],
                        dist_sb[:],
                        mybir.ActivationFunctionType.Sigmoid,
                        bias=bias_sb[:, j : j + 1],
                        scale=-1.0 / SIGMA,
                    )
                    deg = smallp.tile([P, KO], f32, tag="deg")
                    nc.vector.reduce_sum(deg[:], A[:], axis=mybir.AxisListType.X)
                    qdeg = smallp.tile([P, KO], f32, tag="qdeg")
                    nc.vector.tensor_scalar_mul(qdeg[:], deg[:], qs_sb[:, j : j + 1])
                    nc.vector.tensor_scalar_mul(A[:], A[:], qs_sb[:, j : j + 1])
                    dmask = smallp.tile([P, KO, P], f32, tag="dmask")
                    nc.vector.tensor_tensor(
                        dmask[:],
                        ident[:, None, :].to_broadcast([P, KO, P]),
                        qdeg[:, :, None].to_broadcast([P, KO, P]),
                        mybir.AluOpType.mult,
                    )
                    dv = diag_view(A)
                    nc.vector.tensor_tensor(dv, dv, dmask[:], mybir.AluOpType.subtract)

                    # ---- Paterson-Stockmeyer degree-16 Taylor:
                    #   p(A) = ((C3*A4 + C2)*A4 + C1)*A4 + C0,
                    #   Ci = c[4i] I + c[4i+1] A + c[4i+2] A2 + c[4i+3] A3
                    #   (C3 additionally + c[16] A4)
                    A2 = matp.tile([P, KO, N], dt_mm, tag="A2")
                    mm_group(A2, A, A)
                    A3 = matp.tile([P, KO, N], dt_mm, tag="A3")
                    mm_group(A3, A2, A)
                    A4 = matp.tile([P, KO, N], dt_mm, tag="A4")
                    mm_group(A4, A2, A2)

                    def build_C(i, dstC, with_a4=False):
                        if dstC is not A:
                            nc.vector.tensor_scalar_mul(dstC[:], A[:], coef[4 * i + 1])
                        else:  # in-place into A's buffer (A dead afterwards)
                            nc.vector.tensor_scalar_mul(dstC[:], dstC[:], coef[4 * i + 1])
                        nc.vector.scalar_tensor_tensor(
                            dstC[:], A2[:], coef[4 * i + 2], dstC[:],
                            mybir.AluOpType.mult, mybir.AluOpType.add,
                        )
                        nc.vector.scalar_tensor_tensor(
                            dstC[:], A3[:], coef[4 * i + 3], dstC[:],
                            mybir.AluOpType.mult, mybir.AluOpType.add,
                        )
                        if with_a4:
                            nc.vector.scalar_tensor_tensor(
                                dstC[:], A4[:], coef[16], dstC[:],
                                mybir.AluOpType.mult, mybir.AluOpType.add,
                            )
                        diag_add(dstC, cid[4 * i])

                    C3 = matp.tile([P, KO, N], dt_mm, tag="C3")
                    build_C(3, C3, with_a4=True)
                    C2 = matp.tile([P, KO, N], dt_mm, tag="C2")
                    build_C(2, C2)
                    C1 = matp.tile([P, KO, N], dt_mm, tag="C1")
                    build_C(1, C1)
                    build_C(0, A)              # C0 overwrites A (A2/A3 still live)

                    # t1 = C3*A4 + C2 -> A2's buffer (dead);
                    # t2 = t1*A4 + C1 -> A3's buffer (dead after C0 build);
                    # p(A) = t2*A4 + C0 -> squaring pong buffer Sb
                    mm_group(A2, C3, A4, addmat=C2)
                    mm_group(A3, A2, A4, addmat=C1)
                    Sa = sqp.tile([P, KO, N], dt_mm, tag="sq")
                    Sb = sqp.tile([P, KO, N], dt_mm, tag="sq")
                    mm_group(Sb, A3, A4, addmat=A)

                    # ---- (s-1) squarings: n_dsq doubles + n_par single
                    with tc.For_i(0, n_dsq, 1):
                        mm_group(Sa, Sb, Sb)
                        mm_group(Sb, Sa, Sa)
                    with tc.For_i(0, n_par, 1):
                        mm_group(Sa, Sb, Sb)
                        nc.vector.tensor_copy(Sb[:], Sa[:])

                    # ---- frobenius partials: fro[p] = sum_f Sb[p,f]^2
                    # (scratch output aliases Sa, which is dead here)
                    nc.scalar.activation(
                        Sa[:],
                        Sb[:],
                        mybir.ActivationFunctionType.Square,
                        accum_out=fro_sb[:, j : j + 1],
                    )

            nc.gpsimd.dma_start(fro_d.ap(), fro_sb[:])
    nc.compile()
    return nc


def _get_nc(slots):
    key = (USE_FP32R, slots)
    if key not in _COMPILED:
        _COMPILED[key] = _build_nc(slots)
    return _COMPILED[key]


# ---------------------------------------------------------------- entrypoint

def _prepare(points):
    dist = _compute_dist(points)
    max_dist = dist.max()
    thresholds = (np.linspace(0.0, 1.0, NUM_THRESHOLDS).astype(np.float32) * max_dist).astype(np.float32)
    s_arr, trivial = _analyze(dist, thresholds)
    if ALL_DEVICE:
        trivial = np.zeros_like(trivial)
    active = [t for t in range(NUM_THRESHOLDS) if not trivial[t]]
    slots = max(1, -(-len(active) // NCORES))
    assign = _assign(active, s_arr, slots)

    dist_r = np.ascontiguousarray(
        dist.reshape(KO, P, N).transpose(1, 0, 2).reshape(P, KO * N)
    )
    in_maps = []
    for c in range(NCORES):
        ts = assign[c]
        bias = np.tile((thresholds[ts] / SIGMA)[None, :], (P, 1)).astype(np.float32)
        qs = np.tile(
            np.array([1.0 / (SIGMA * 2.0 ** int(s_arr[t])) for t in ts], dtype=np.float32)[None, :],
            (P, 1),
        )
        nsq = np.array([[(int(s_arr[t]) - 1) // 2 for t in ts]], dtype=np.int32)
        npar = np.array([[(int(s_arr[t]) - 1) % 2 for t in ts]], dtype=np.int32)
        in_maps.append({
            "dist": dist_r, "bias": bias, "qs": qs, "nsq": nsq, "npar": npar,
            "nrep": np.array([[1]], dtype=np.int32),
        })
    return thresholds, s_arr, trivial, slots, assign, in_maps


def kernel(points):
    from concourse.bass_utils import run_bass_kernel_spmd

    thresholds, s_arr, trivial, slots, assign, in_maps = _prepare(points)
    nc = _get_nc(slots)
    res = run_bass_kernel_spmd(nc, in_maps, list(range(NCORES)))
    betti = np.ones(NUM_THRESHOLDS, dtype=np.float64)  # trivial thresholds: exactly 1
    for c in range(NCORES):
        fro = res.results[c]["fro"]
        for j in range(slots):
            betti[assign[c][j]] = fro[:, j].sum(dtype=np.float64)
    LAST_BETTI = betti.copy()
    return _landscapes(betti)


LAST_BETTI = None


# revision 33
# speedup vs baseline: 1.1248x; 1.1248x over previous
"""Trainium2 Bass kernel for nn_DifferentiablePersistence.

Math: betti_0(t) = sum_i exp(-lambda_i(L_t)/sigma) = trace(expm(-L_t/sigma)),
so no eigensolver is needed -- a scaling-and-squaring matrix exponential
(pure 768^3 fp32 matmuls on the tensor engine) computes the trace:

    A     = -(L/sigma) / 2^s           (symmetric NSD, ||A|| <= THETA)
    B     = taylor_16(A)               (Paterson-Stockmeyer: 6 matmuls,
                                        coefficient blocks on the vector engine)
    B    <- B^2, (s-1) times           (chained squarings)
    betti = ||B||_F^2                  (= trace(B^(2^s)), last squaring free)

Host-side spectral triage (power iterations, no eigensolves): thresholds
whose algebraic connectivity lam_2 >= 3 have betti = 1 + 767*exp(-30) --
exactly 1 to 7 digits -- and skip the device entirely.  The remaining
(~13) thresholds are LPT-balanced over 8 cores x ceil(K/8) slots.  The
per-threshold squaring count s (from lambda_max(L_t)) is runtime data
driving a register-bound hardware loop, so one SPMD NEFF serves every
threshold with its exact s.  The distance matrix is replicated; each core
returns per-slot Frobenius partials (ACT Square + per-channel accumulate)
which the host reduces and feeds through the cheap (5,100) landscape
post-processing.
"""

import math
import os

import numpy as np

SIGMA = 0.1
RESOLUTION = 100
NUM_LANDSCAPES = 5
NUM_THRESHOLDS = 50
N = 768
P = 128
KO = N // P          # 6 k-subtiles
NCHUNK = 384         # matmul free-dim chunk (>=256 keeps float32r at rate 1.0)
NCORES = 8
SLOTS = 7
DEG = 16             # Taylor degree (Paterson-Stockmeyer blocks of 4)
THETA = 1.8          # max ||A|| after scaling (deg-16 remainder < 1e-10)

# build-time switches (env overrides for experiments)
USE_FP32R = os.environ.get("KB_FP32R", "0") == "1"
MAX_DSQ = 8          # upper bound on double-squaring trip count (s <= 17)
LAM2_TRIVIAL = 2.0   # lam_2 above this => betti-1 <= 767*exp(-20) ~ 4e-6: negligible
                     # vs the landscape scale (betti max ~200); skip device work
ALL_DEVICE = os.environ.get("KB_ALL_DEVICE", "0") == "1"

_COMPILED = {}


# ----------------------------------------------------------------- host math

def _compute_dist(points):
    """fp32 pairwise distances exactly like the jax reference."""
    pts = points.astype(np.float32)
    diff = pts[:, None, :] - pts[None, :, :]
    d2 = (diff * diff).sum(-1, dtype=np.float32)
    dist = np.where(d2 > 0, np.sqrt(np.where(d2 > 0, d2, np.float32(1.0))), np.float32(0.0))
    return dist.astype(np.float32)


def _analyze(dist, thresholds):
    """Per-threshold spectral analysis via power iterations (host, cheap):

    - lam_max(L_t)  -> squaring count s_t
    - lam_2(L_t)    -> thresholds with lam_2 >= LAM2_TRIVIAL are 'trivial':
      betti-1 = sum_{i>=2} exp(-lam_i/sigma) <= N*exp(-LAM2_TRIVIAL/sigma)
      < 2e-7, so betti := 1 exactly (the lam=0 constant mode survives).

    Returns (s_arr, trivial_mask)."""
    T = len(thresholds)
    d = dist.astype(np.float32)
    S = 1.0 / (1.0 + np.exp(-(thresholds[:, None, None].astype(np.float32) - d) / np.float32(SIGMA)))
    deg = S.sum(-1)                                     # (T, N)

    def power_lmax(niter=40):
        v = deg / np.linalg.norm(deg, axis=-1, keepdims=True)
        lam = np.zeros(T)
        for _ in range(niter):
            w = deg * v - np.einsum("tij,tj->ti", S, v)  # L v
            lam = np.abs((v * w).sum(-1))
            v = w / np.maximum(np.linalg.norm(w, axis=-1, keepdims=True), 1e-30)
        return lam

    lam_max = power_lmax()
    lub = lam_max * 1.08 + 1e-6
    a = lub / SIGMA
    s = np.ceil(np.log2(np.maximum(a / THETA, 1.0))).astype(np.int64)
    s = np.maximum(s, 1)
    s = np.minimum(s, 2 * MAX_DSQ + 1)

    # lam_2 via power iteration on M = lub*I - L restricted to 1-perp
    rng = np.random.default_rng(12345)
    lam2_ests = []
    for _ in range(2):
        v = rng.standard_normal((T, dist.shape[0])).astype(np.float64)
        v -= v.mean(-1, keepdims=True)
        v /= np.linalg.norm(v, axis=-1, keepdims=True)
        top = np.zeros(T)
        for _ in range(80):
            Lv = deg * v - np.einsum("tij,tj->ti", S, v)
            w = lub[:, None] * v - Lv                    # M v
            w -= w.mean(-1, keepdims=True)               # project out constant
            top = (v * w).sum(-1)
            v = w / np.maximum(np.linalg.norm(w, axis=-1, keepdims=True), 1e-30)
        lam2_ests.append(lub - top)                      # >= lam_2 (upper est)
    lam2 = np.minimum(*lam2_ests)
    trivial = lam2 >= LAM2_TRIVIAL
    return s, trivial


def _assign(active, s_arr, slots):
    """LPT-balance the active thresholds + duplicate pad items onto
    (core, slot) by per-item cost (DEG-1 Horner matmuls + s squarings).

    Returns assign[core][slot] = threshold index."""
    npad = NCORES * slots - len(active)
    cheap = sorted(active, key=lambda t: s_arr[t])[:npad]
    while len(cheap) < npad:  # more pads than actives: repeat
        cheap = (cheap + cheap)[:npad]
    items = list(active) + cheap
    items.sort(key=lambda t: -s_arr[t])
    loads = [0.0] * NCORES
    counts = [0] * NCORES
    assign = [[] for _ in range(NCORES)]
    for t in items:
        c = min(
            (c for c in range(NCORES) if counts[c] < slots),
            key=lambda c: (loads[c], counts[c]),
        )
        assign[c].append(int(t))
        loads[c] += (DEG - 1) + int(s_arr[t])
        counts[c] += 1
    return assign


def _landscapes(betti_0):
    """Replicate the reference post-processing (host side, float64)."""
    x = betti_0.astype(np.float64)
    t = x.shape[0]
    pos = np.linspace(0.0, t - 1.0, RESOLUTION)
    i0 = np.clip(np.floor(pos).astype(np.int64), 0, t - 2)
    frac = pos - i0
    bi = x[i0] * (1.0 - frac) + x[i0 + 1] * frac
    out = [bi / (bi.max() + 1e-8)]
    for k in range(1, NUM_LANDSCAPES):
        ks = min(2 * k + 1, RESOLUTION // 4)
        if ks > 1:
            pad = ks // 2
            padded = np.pad(bi, (pad, pad), mode="edge")
            sm = np.convolve(padded, np.ones(ks) / ks, mode="valid")
            dv = sm[1:] - sm[:-1]
            dv = np.concatenate([dv, dv[-1:]])
            out.append(dv / (np.abs(dv).max() + 1e-8))
        else:
            out.append(out[0])
    return np.stack(out).astype(np.float32)


# -------------------------------------------------------------- bass kernel

def _build_nc(slots):
    import concourse.bass as bass
    import concourse.mybir as mybir
    import concourse.tile as tile
    from concourse import bacc
    from concourse.masks import make_identity

    f32 = mybir.dt.float32
    dt_mm = mybir.dt.float32r if USE_FP32R else mybir.dt.float32

    nc = bacc.Bacc("TRN2", target_bir_lowering=False)
    dist_d = nc.declare_dram_parameter("dist", [P, KO * N], f32, isOutput=False)
    bias_d = nc.declare_dram_parameter("bias", [P, slots], f32, isOutput=False)
    qs_d = nc.declare_dram_parameter("qs", [P, slots], f32, isOutput=False)
    nsq_d = nc.declare_dram_parameter("nsq", [1, slots], mybir.dt.int32, isOutput=False)
    npar_d = nc.declare_dram_parameter("npar", [1, slots], mybir.dt.int32, isOutput=False)
    nrep_d = nc.declare_dram_parameter("nrep", [1, 1], mybir.dt.int32, isOutput=False)
    fro_d = nc.declare_dram_parameter("fro", [P, slots], f32, isOutput=True)

    coef = [1.0 / math.factorial(k) for k in range(DEG + 1)]

    with tile.TileContext(nc) as tc:
        with (
            tc.tile_pool(name="const", bufs=1) as constp,
            tc.tile_pool(name="mats", bufs=1) as matp,
            tc.tile_pool(name="sq", bufs=2) as sqp,
            tc.tile_pool(name="small", bufs=2) as smallp,
            tc.tile_pool(name="ps", bufs=8, space="PSUM") as psp,
        ):
            dist_sb = constp.tile([P, KO, N], f32, tag="dist")
            nc.gpsimd.dma_start(dist_sb[:], dist_d.ap().rearrange("p (ko f) -> p ko f", ko=KO))
            bias_sb = constp.tile([P, slots], f32, tag="bias")
            nc.gpsimd.dma_start(bias_sb[:], bias_d.ap())
            qs_sb = constp.tile([P, slots], f32, tag="qs")
            nc.gpsimd.dma_start(qs_sb[:], qs_d.ap())
            nsq_sb = constp.tile([1, slots], mybir.dt.int32, tag="nsq")
            nc.gpsimd.dma_start(nsq_sb[:], nsq_d.ap())
            npar_sb = constp.tile([1, slots], mybir.dt.int32, tag="npar")
            nc.gpsimd.dma_start(npar_sb[:], npar_d.ap())
            nrep_sb = constp.tile([1, 1], mybir.dt.int32, tag="nrep")
            nc.gpsimd.dma_start(nrep_sb[:], nrep_d.ap())

            ident = constp.tile([P, P], f32, tag="ident")
            make_identity(nc, ident[:])
            cid = []
            for k in range(DEG):                      # scaled identities coef[0..11]
                ck = constp.tile([P, P], f32, tag=f"cid{k}")
                nc.vector.tensor_scalar_mul(ck[:], ident[:], coef[k])
                cid.append(ck)

            fro_sb = constp.tile([P, slots], f32, tag="fro")

            def mm_group(dst, lhs, rhs, addmat=None):
                """dst = lhs @ rhs (+ addmat) -- [P, KO, N] fp32 SBUF, symmetric
                operands; evacuation alternates DVE/ACT, or does a DVE
                PSUM+SBUF add when addmat is given."""
                piece = 0
                for m in range(KO):
                    for n0 in range(0, N, NCHUNK):
                        pt = psp.tile([P, NCHUNK], f32, tag="ps")
                        for k in range(KO):
                            nc.tensor.matmul(
                                pt[:],
                                lhs[:, k, m * P : (m + 1) * P],
                                rhs[:, k, n0 : n0 + NCHUNK],
                                start=(k == 0),
                                stop=(k == KO - 1),
                            )
                        out_ap = dst[:, m, n0 : n0 + NCHUNK]
                        if addmat is not None:
                            nc.vector.tensor_tensor(
                                out_ap, pt[:], addmat[:, m, n0 : n0 + NCHUNK],
                                mybir.AluOpType.add,
                            )
                        elif piece % 2 == 0:
                            nc.vector.tensor_copy(out_ap, pt[:])
                        else:
                            nc.scalar.copy(out_ap, pt[:])
                        piece += 1

            def diag_view(mat):
                """[P, KO, P] view of mat's 128-block diagonal: (p, ko, f) ->
                free offset ko*(N+P) + f."""
                t = mat[:]
                return bass.AP(t.tensor, t.offset, [[KO * N, P], [N + P, KO], [1, P]])

            def diag_add(mat, ck):
                """mat[i, i] += ck[i % P, i % P] (per 128-block diagonal)."""
                dv = diag_view(mat)
                nc.vector.tensor_tensor(
                    dv,
                    dv,
                    ck[:, None, :].to_broadcast([P, KO, P]),
                    mybir.AluOpType.add,
                )

            rep_regs = []
            for e in mybir.ALL_ENGINES:
                r = nc.alloc_register(e, f"nrep_{e.name}")
                nc.engines[e].reg_load(r, nrep_sb[:1, :1])
                rep_regs.append(r)
            n_rep = bass.make_scalar_value(
                bass.RegisterHandles(rep_regs), min_val=1, max_val=1000000
            )

            with tc.For_i(0, n_rep, 1):
                for j in range(slots):
                    # ---- per-slot registers: double-squaring trip count
                    regs = []
                    for e in mybir.ALL_ENGINES:
                        r = nc.alloc_register(e, f"nsq_{j}_{e.name}")
                        nc.engines[e].reg_load(r, nsq_sb[:1, j : j + 1])
                        regs.append(r)
                    n_dsq = bass.make_scalar_value(
                        bass.RegisterHandles(regs), min_val=0, max_val=MAX_DSQ
                    )
                    regs_p = []
                    for e in mybir.ALL_ENGINES:
                        r = nc.alloc_register(e, f"npar_{j}_{e.name}")
                        nc.engines[e].reg_load(r, npar_sb[:1, j : j + 1])
                        regs_p.append(r)
                    n_par = bass.make_scalar_value(
                        bass.RegisterHandles(regs_p), min_val=0, max_val=1
                    )

                    # ---- A = qs * (S - diag(deg));  S = sigmoid(-dist/sigma + t/sigma)
                    A = matp.tile([P, KO, N], dt_mm, tag="A")
                    nc.scalar.activation(
                        A[:],
                        dist_sb[:],
                        mybir.ActivationFunctionType.Sigmoid,
                        bias=bias_sb[:, j : j + 1],
                        scale=-1.0 / SIGMA,
                    )
                    deg = smallp.tile([P, KO], f32, tag="deg")
                    nc.vector.reduce_sum(deg[:], A[:], axis=mybir.AxisListType.X)
                    qdeg = smallp.tile([P, KO], f32, tag="qdeg")
                    nc.vector.tensor_scalar_mul(qdeg[:], deg[:], qs_sb[:, j : j + 1])
                    nc.vector.tensor_scalar_mul(A[:], A[:], qs_sb[:, j : j + 1])
                    dmask = smallp.tile([P, KO, P], f32, tag="dmask")
                    nc.vector.tensor_tensor(
                        dmask[:],
                        ident[:, None, :].to_broadcast([P, KO, P]),
                        qdeg[:, :, None].to_broadcast([P, KO, P]),
                        mybir.AluOpType.mult,
                    )
                    dv = diag_view(A)
                    nc.vector.tensor_tensor(dv, dv, dmask[:], mybir.AluOpType.subtract)

                    # ---- Paterson-Stockmeyer degree-16 Taylor:
                    #   p(A) = ((C3*A4 + C2)*A4 + C1)*A4 + C0,
                    #   Ci = c[4i] I + c[4i+1] A + c[4i+2] A2 + c[4i+3] A3
                    #   (C3 additionally + c[16] A4)
                    A2 = matp.tile([P, KO, N], dt_mm, tag="A2")
                    mm_group(A2, A, A)
                    A3 = matp.tile([P, KO, N], dt_mm, tag="A3")
                    mm_group(A3, A2, A)
                    A4 = matp.tile([P, KO, N], dt_mm, tag="A4")
                    mm_group(A4, A2, A2)

                    def build_C(i, dstC, with_a4=False):
                        if dstC is not A:
                            nc.vector.tensor_scalar_mul(dstC[:], A[:], coef[4 * i + 1])
                        else:  # in-place into A's buffer (A dead afterwards)
                            nc.vector.tensor_scalar_mul(dstC[:], dstC[:], coef[4 * i + 1])
                        nc.vector.scalar_tensor_tensor(
                            dstC[:], A2[:], coef[4 * i + 2], dstC[:],
                            mybir.AluOpType.mult, mybir.AluOpType.add,
                        )
                        nc.vector.scalar_tensor_tensor(
                            dstC[:], A3[:], coef[4 * i + 3], dstC[:],
                            mybir.AluOpType.mult, mybir.AluOpType.add,
                        )
                        if with_a4:
                            nc.vector.scalar_tensor_tensor(
                                dstC[:], A4[:], coef[16], dstC[:],
                                mybir.AluOpType.mult, mybir.AluOpType.add,
                            )
                        diag_add(dstC, cid[4 * i])

                    C3 = matp.tile([P, KO, N], dt_mm, tag="C3")
                    build_C(3, C3, with_a4=True)
                    C2 = matp.tile([P, KO, N], dt_mm, tag="C2")
                    build_C(2, C2)
                    C1 = matp.tile([P, KO, N], dt_mm, tag="C1")
                    build_C(1, C1)
                    build_C(0, A)              # C0 overwrites A (A2/A3 still live)

                    # t1 = C3*A4 + C2 -> A2's buffer (dead);
                    # t2 = t1*A4 + C1 -> A3's buffer (dead after C0 build);
                    # p(A) = t2*A4 + C0 -> squaring pong buffer Sb
                    mm_group(A2, C3, A4, addmat=C2)
                    mm_group(A3, A2, A4, addmat=C1)
                    Sa = sqp.tile([P, KO, N], dt_mm, tag="sq")
                    Sb = sqp.tile([P, KO, N], dt_mm, tag="sq")
                    mm_group(Sb, A3, A4, addmat=A)

                    # ---- (s-1) squarings: n_dsq doubles + n_par single
                    with tc.For_i(0, n_dsq, 1):
                        mm_group(Sa, Sb, Sb)
                        mm_group(Sb, Sa, Sa)
                    with tc.For_i(0, n_par, 1):
                        mm_group(Sa, Sb, Sb)
                        nc.vector.tensor_copy(Sb[:], Sa[:])

                    # ---- frobenius partials: fro[p] = sum_f Sb[p,f]^2
                    # (scratch output aliases Sa, which is dead here)
                    nc.scalar.activation(
                        Sa[:],
                        Sb[:],
                        mybir.ActivationFunctionType.Square,
                        accum_out=fro_sb[:, j : j + 1],
                    )

            nc.gpsimd.dma_start(fro_d.ap(), fro_sb[:])
    nc.compile()
    return nc


def _get_nc(slots):
    key = (USE_FP32R, slots)
    if key not in _COMPILED:
        _COMPILED[key] = _build_nc(slots)
    return _COMPILED[key]


# ---------------------------------------------------------------- entrypoint

def _prepare(points):
    dist = _compute_dist(points)
    max_dist = dist.max()
    thresholds = (np.linspace(0.0, 1.0, NUM_THRESHOLDS).astype(np.float32) * max_dist).astype(np.float32)
    s_arr, trivial = _analyze(dist, thresholds)
    if ALL_DEVICE:
        trivial = np.zeros_like(trivial)
    active = [t for t in range(NUM_THRESHOLDS) if not trivial[t]]
    slots = max(1, -(-len(active) // NCORES))
    assign = _assign(active, s_arr, slots)

    dist_r = np.ascontiguousarray(
        dist.reshape(KO, P, N).transpose(1, 0, 2).reshape(P, KO * N)
    )
    in_maps = []
    for c in range(NCORES):
        ts = assign[c]
        bias = np.tile((thresholds[ts] / SIGMA)[None, :], (P, 1)).astype(np.float32)
        qs = np.tile(
            np.array([1.0 / (SIGMA * 2.0 ** int(s_arr[t])) for t in ts], dtype=np.float32)[None, :],
            (P, 1),
        )
        nsq = np.array([[(int(s_arr[t]) - 1) // 2 for t in ts]], dtype=np.int32)
        npar = np.array([[(int(s_arr[t]) - 1) % 2 for t in ts]], dtype=np.int32)
        in_maps.append({
            "dist": dist_r, "bias": bias, "qs": qs, "nsq": nsq, "npar": npar,
            "nrep": np.array([[1]], dtype=np.int32),
        })
    return thresholds, s_arr, trivial, slots, assign, in_maps


def kernel(points):
    from concourse.bass_utils import run_bass_kernel_spmd

    thresholds, s_arr, trivial, slots, assign, in_maps = _prepare(points)
    nc = _get_nc(slots)
    res = run_bass_kernel_spmd(nc, in_maps, list(range(NCORES)))
    betti = np.ones(NUM_THRESHOLDS, dtype=np.float64)  # trivial thresholds: exactly 1
    for c in range(NCORES):
        fro = res.results[c]["fro"]
        for j in range(slots):
            betti[assign[c][j]] = fro[:, j].sum(dtype=np.float64)
    LAST_BETTI = betti.copy()
    return _landscapes(betti)


LAST_BETTI = None
